# revision 8
# baseline (speedup 1.0000x reference)
"""Trainium2 Bass kernel for the BasicRNN problem.

Math: the reference's 9 block matmuls per step collapse to
    state_{t+1} = relu(state_t @ W + gate_t * [E, 0, 0]),  state [256, 4096]
with E = x @ in_w.T + in_b, gates at t % 5 == 0, output = O_T @ out_w.T + out_b
where O is the last 1024 state columns.

Device strategy (8 cores, tensor-parallel over W columns):
- Keep the state transposed on-chip: sT = state.T [4096, 256], stored as 32
  row-chunks of 128. W stays stationary on the PE; each step streams sT
  through it: next_sT[rows r] = W[:, r].T @ sT.
- The 4096 state rows are block-permuted so core c owns device blocks
  4c..4c+3 = original chunks {S_c, I_2c, I_2c+1, O_c}. Every core then owns
  one S chunk (balanced E injection) and one O chunk (balanced last step,
  which only needs O). Step 1 contracts only over the 8 S chunks (state_1 is
  [relu(E), 0, 0]).
- Matmul operands are bf16 (fp32 matmul is 4x slower; fp32/f32r weight loads
  are 2x slower). PSUM accumulates fp32. Measured rel err ~7e-3.
- Per step the 4 output blocks are split in two halves; each half is
  AllGathered separately so the collective of one half overlaps the matmuls
  of the other half and of the next step (k-loop runs H0 chunks before H1).
- A tiny throwaway AllGather at program start absorbs the one-time
  communicator-init barrier (~34us) under the weight loads.
- Filler matmuls (identity x E) occupy the PE during collective waits so the
  HAM clock gate stays at 2.4 GHz (idle >3.4us would re-throttle to 1.2).
- State reloads are split across the two HWDGE rings (sync + scalar).
- Final classifier is sharded: core c computes out.T rows [125c, 125c+125)
  with bias; host concatenates and transposes.
"""

import numpy as np

S_DIM, I_DIM, O_DIM = 1024, 2048, 1024
TOTAL = 4096
INPUT_DIM, NUM_CLASSES, BATCH = 2048, 1000, 256
NC = 8
KC = TOTAL // 128          # 32 k-chunks of 128
MPC = KC // NC             # 4 blocks per core
CLS_M = NUM_CLASSES // NC  # 125 classifier rows per core

# device block -> original 128-row chunk of the state vector
BLOCK_PERM = []
for c in range(NC):
    BLOCK_PERM += [c, 8 + 2 * c, 9 + 2 * c, 24 + c]

FILL_WARM = 60   # filler matmuls after the E phase (covers AG0)
FILL_A = 90      # fillers between phase B(s) and phase A(s+1) (covers AG_a)
FILL_B = 50      # fillers between phase A(s) and phase B(s) (covers AG_b)

_cache = {}


def _split_excess_waits(nc, mybir, limit=1, nop_limit=1):
    """This walrus build rejects multiple sync-waits on most instruction
    structs and any wait on Drain. Move excess waits onto preceding
    same-engine nops."""
    counter = [0]

    def make_nop(engine, waits):
        counter[0] += 1
        nop = mybir.InstNoOp(name=f"I-ws{counter[0]}", engine=engine)
        nop.sync_info = mybir.SyncInfo(on_wait=list(waits), on_update=[])
        return nop

    for fn in nc.m.functions:
        for bb in fn.blocks:
            out = []
            changed = False
            for inst in bb.instructions:
                si = getattr(inst, "sync_info", None)
                waits = list(si.on_wait) if si is not None and si.on_wait else []
                lim = 0 if isinstance(inst, mybir.InstDrain) else limit
                if len(waits) > lim:
                    keep = waits[-lim:] if lim else []
                    excess = waits[: len(waits) - lim]
                    for g in range(0, len(excess), nop_limit):
                        nop = make_nop(inst.engine, excess[g : g + nop_limit])
                        nc.register_instruction(nop, overwrite=True)
                        out.append(nop)
                    si.on_wait = keep
                    changed = True
                out.append(inst)
            if changed:
                bb.instructions = out


def _build(T):
    import concourse.bass as bass
    import concourse.tile as tile
    from concourse import mybir

    f32 = mybir.dt.float32
    bf16 = mybir.dt.bfloat16
    B = BATCH

    nc = bass.Bass()
    wc = nc.dram_tensor("wc", [TOTAL, 128 * MPC], bf16, kind="ExternalInput")
    xT = nc.dram_tensor("xT", [INPUT_DIM, B], bf16, kind="ExternalInput")
    inwTc = nc.dram_tensor("inwTc", [INPUT_DIM, 128], bf16, kind="ExternalInput")
    inbc = nc.dram_tensor("inbc", [128, 1], f32, kind="ExternalInput")
    outwTc = nc.dram_tensor("outwTc", [O_DIM, CLS_M], bf16, kind="ExternalInput")
    outbc = nc.dram_tensor("outbc", [CLS_M, 1], f32, kind="ExternalInput")
    ident = nc.dram_tensor("ident", [128, 128], bf16, kind="ExternalInput")
    out_t = nc.dram_tensor("out_t", [CLS_M, B], f32, kind="ExternalOutput")

    S_BLOCKS = [4 * c for c in range(NC)]  # device blocks holding S chunks
    RG = [list(range(NC))]
    ROW_A = {k: 256 * (k // 4) + 128 * (k % 4) for k in range(KC) if k % 4 < 2}
    ROW_B = {k: 256 * (k // 4) + 128 * (k % 4 - 2) for k in range(KC) if k % 4 >= 2}

    def ag(ins_ap, out_ap):
        nc.gpsimd.collective_compute(
            "AllGather", mybir.AluOpType.bypass,
            replica_groups=RG, ins=[ins_ap], outs=[out_ap],
        )

    with tile.TileContext(nc) as tc:
        with (
            tc.tile_pool(name="wp", bufs=1) as wp,
            tc.tile_pool(name="pers", bufs=1) as pers,
            tc.tile_pool(name="state", bufs=2) as stp,
            tc.tile_pool(name="res", bufs=2) as resp,
            tc.tile_pool(name="psum", bufs=2, space="PSUM") as psp,
            tc.tile_pool(name="psum1", bufs=1, space="PSUM") as psp1,
            tc.tile_pool(name="dram", bufs=2, space="DRAM") as dram,
        ):
            # throwaway collective: pays the one-time communicator barrier
            # while the weight DMAs run
            dmy_in = dram.tile([128, 8], bf16, name="dmyin", tag="dmyin")
            nc.sync.dma_start(dmy_in[:], ident[:, 0:8])
            dmy_out = dram.tile(
                [1024, 8], bf16, name="dmyout", tag="dmyout", addr_space="Shared"
            )
            ag(dmy_in.opt(), dmy_out.opt())

            # --- static weights/consts into SBUF (packed single DMAs) ---
            wt = wp.tile([128, KC * 512], bf16, name="wt", tag="wt")
            nc.sync.dma_start(
                wt[:].rearrange("p (k m) -> p k m", m=512),
                wc[:].rearrange("(k p) m -> p k m", p=128),
            )

            def wslice(k, m):  # lhsT tile [128, 128] for (k-chunk, m-block)
                return wt[:, k * 512 + 128 * m : k * 512 + 128 * (m + 1)]

            xt = pers.tile([128, (INPUT_DIM // 128) * B], bf16, name="xt", tag="xt")
            nc.scalar.dma_start(
                xt[:].rearrange("p (k b) -> p k b", b=B),
                xT[:].rearrange("(k p) b -> p k b", p=128),
            )
            iwt = pers.tile([128, INPUT_DIM], bf16, name="iwt", tag="iwt")
            nc.scalar.dma_start(
                iwt[:].rearrange("p (k m) -> p k m", m=128),
                inwTc[:].rearrange("(k p) m -> p k m", p=128),
            )
            id_t = pers.tile([128, 128], bf16, name="ident", tag="ident")
            nc.scalar.dma_start(id_t[:], ident[:])
            inb_t = pers.tile([128, 1], f32, name="inb", tag="inb")
            nc.scalar.dma_start(inb_t[:], inbc[:])
            outb_t = pers.tile([CLS_M, 1], f32, name="outb", tag="outb")
            nc.scalar.dma_start(outb_t[:], outbc[:])
            owt = pers.tile([128, (O_DIM // 128) * CLS_M], bf16, name="owt", tag="owt")
            nc.scalar.dma_start(
                owt[:].rearrange("p (k m) -> p k m", m=CLS_M),
                outwTc[:].rearrange("(k p) m -> p k m", p=128),
            )

            # --- E phase: einj = (in_w @ x.T)[own chunk] + in_b  [128, B] ---
            ps_e = psp.tile([128, B], f32, name="ps0", tag="ps0")
            for k in range(INPUT_DIM // 128):
                nc.tensor.matmul(
                    ps_e[:],
                    iwt[:, 128 * k : 128 * (k + 1)],
                    xt[:, B * k : B * (k + 1)],
                    start=(k == 0), stop=(k == INPUT_DIM // 128 - 1),
                )
            einj = pers.tile([128, B], bf16, name="einj", tag="einj")
            nc.scalar.activation(
                einj[:], ps_e[:], mybir.ActivationFunctionType.Identity, bias=inb_t[:]
            )
            st1_own = resp.tile([128, B], bf16, name="res0", tag="res0")
            nc.scalar.activation(
                st1_own[:], ps_e[:], mybir.ActivationFunctionType.Relu, bias=inb_t[:]
            )
            zero_t = pers.tile([128, B], bf16, name="zero", tag="zero")
            nc.vector.tensor_scalar_mul(zero_t[:], st1_own[:], 0.0)

            ps_d = psp1.tile([128, B], f32, name="psd", tag="psd")

            def fill(n, rhs=None):  # keep PE busy/warm through a collective wait
                # rhs pins the fillers to the producing step: without a
                # data dep the scheduler drains every filler into the first
                # simulated bubble (its collective cost model is ~3x real).
                r = einj if rhs is None else rhs
                for _ in range(n):
                    nc.tensor.matmul(ps_d[:], id_t[:], r[:, 0:B], start=True, stop=True)

            # t0: sT_1 = [relu(E), 0, 0]; step 1 reads only S blocks (all in
            # the A half), so a single half-gather suffices.
            ag_a = dram.tile([256, B], bf16, name="aga", tag="aga")
            nc.sync.dma_start(ag_a[0:128, :], st1_own[:])
            nc.sync.dma_start(ag_a[128:256, :], zero_t[:])
            agout_a = dram.tile(
                [2048, B], bf16, name="agouta", tag="agouta", addr_space="Shared"
            )
            ag(ag_a.opt(), agout_a.opt())
            agout_b = None
            fill(FILL_WARM)

            # --- RNN steps s = 1..T-1: sT_{s+1} = relu(W.T @ sT_s [+ inj]) ---
            last = T - 1
            res = {}
            for s in range(1, T):
                active = S_BLOCKS if s == 1 else list(range(KC))
                m_list = [MPC - 1] if s == last else list(range(MPC))
                act_a = [k for k in active if k % 4 < 2]
                act_b = [k for k in active if k % 4 >= 2]
                inject = s % 5 == 0

                st = {}
                for j, k in enumerate(act_a):
                    st[k] = stp.tile([128, B], bf16, name=f"st{k}", tag=f"st{k}")
                    eng = nc.sync if j % 2 == 0 else nc.scalar
                    eng.dma_start(st[k][:], agout_a[ROW_A[k] : ROW_A[k] + 128, :])
                for j, k in enumerate(act_b):
                    st[k] = stp.tile([128, B], bf16, name=f"st{k}", tag=f"st{k}")
                    eng = nc.sync if j % 2 == 0 else nc.scalar
                    eng.dma_start(st[k][:], agout_b[ROW_B[k] : ROW_B[k] + 128, :])

                # phase A: H0 contraction for every output block
                ps = {}
                for m in m_list:
                    pool = psp1 if m == MPC - 1 else psp
                    ps[m] = pool.tile([128, B], f32, name=f"ps{m}", tag=f"ps{m}")
                    for i, k in enumerate(act_a):
                        nc.tensor.matmul(
                            ps[m][:],
                            wslice(k, m),
                            st[k][:],
                            start=(i == 0),
                            stop=(not act_b)
                            and i == len(act_a) - 1
                            and not (inject and m == 0),
                        )
                if act_b:
                    fill(FILL_B, rhs=st[act_a[-1]])
                # phase B: H1 contraction, relu, shard out, half-gathers
                res = {}
                for m in m_list:
                    for i, k in enumerate(act_b):
                        nc.tensor.matmul(
                            ps[m][:],
                            wslice(k, m),
                            st[k][:],
                            start=False,
                            stop=i == len(act_b) - 1 and not (inject and m == 0),
                        )
                    if inject and m == 0:
                        nc.tensor.matmul(
                            ps[m][:], id_t[:], einj[:], start=False, stop=True
                        )
                    res[m] = resp.tile([128, B], bf16, name=f"res{m}", tag=f"res{m}")
                    nc.vector.tensor_relu(res[m][:], ps[m][:])
                    if s == last:
                        continue
                    if m == 1:
                        new_a = dram.tile([256, B], bf16, name="aga", tag="aga")
                        nc.sync.dma_start(new_a[0:128, :], res[0][:])
                        nc.sync.dma_start(new_a[128:256, :], res[1][:])
                        agout_a = dram.tile(
                            [2048, B], bf16, name="agouta", tag="agouta",
                            addr_space="Shared",
                        )
                        ag(new_a.opt(), agout_a.opt())
                    if m == 3:
                        new_b = dram.tile([256, B], bf16, name="agb", tag="agb")
                        nc.sync.dma_start(new_b[0:128, :], res[2][:])
                        nc.sync.dma_start(new_b[128:256, :], res[3][:])
                        agout_b = dram.tile(
                            [2048, B], bf16, name="agoutb", tag="agoutb",
                            addr_space="Shared",
                        )
                        ag(new_b.opt(), agout_b.opt())
                if s != last:
                    fill(FILL_A, rhs=res[MPC - 1])

            # --- gather O chunks, classifier slice, bias, out ---
            ago_in = dram.tile([128, B], bf16, name="agoin", tag="agoin")
            nc.sync.dma_start(ago_in[:], res[MPC - 1][:])
            ago_out = dram.tile(
                [O_DIM, B], bf16, name="agoout", tag="agoout", addr_space="Shared"
            )
            ag(ago_in.opt(), ago_out.opt())
            fill(FILL_A, rhs=res[MPC - 1])
            ot = {}
            for k in range(O_DIM // 128):
                ot[k] = stp.tile([128, B], bf16, name=f"ot{k}", tag=f"st{k}")
                eng = nc.sync if k % 2 == 0 else nc.scalar
                eng.dma_start(ot[k][:], ago_out[128 * k : 128 * (k + 1), :])
            ps_c = psp.tile([CLS_M, B], f32, name="ps1", tag="ps1")
            for k in range(O_DIM // 128):
                nc.tensor.matmul(
                    ps_c[:], owt[:, CLS_M * k : CLS_M * (k + 1)], ot[k][:],
                    start=(k == 0), stop=(k == O_DIM // 128 - 1),
                )
            out_sb = pers.tile([CLS_M, B], f32, name="outsb", tag="outsb")
            nc.scalar.activation(
                out_sb[:], ps_c[:], mybir.ActivationFunctionType.Identity,
                bias=outb_t[:],
            )
            nc.sync.dma_start(out_t[:], out_sb[:])

    _split_excess_waits(nc, mybir)
    return nc


def kernel(x, W, in_w, in_b, out_w, out_b, time_steps):
    T = int(time_steps)
    x = np.ascontiguousarray(x, dtype=np.float32)
    W = np.ascontiguousarray(W, dtype=np.float32)
    in_w = np.ascontiguousarray(in_w, dtype=np.float32)
    in_b = np.ascontiguousarray(in_b, dtype=np.float32)
    out_w = np.ascontiguousarray(out_w, dtype=np.float32)
    out_b = np.ascontiguousarray(out_b, dtype=np.float32)

    if T < 2:
        # T=0: O stays 0; T=1: state_1 = [relu(E),0,0], O still 0.
        return np.broadcast_to(out_b, (BATCH, NUM_CLASSES)).astype(np.float32).copy()

    import ml_dtypes
    from concourse.bass_utils import run_bass_kernel_spmd

    if T not in _cache:
        _cache[T] = _build(T)
    nc = _cache[T]

    bf = ml_dtypes.bfloat16
    # block-permute W rows and columns to the device layout
    Wd = W.reshape(KC, 128, TOTAL)[BLOCK_PERM].reshape(TOTAL, TOTAL)
    Wd = Wd.reshape(TOTAL, KC, 128)[:, BLOCK_PERM].reshape(TOTAL, TOTAL)
    Wd = Wd.astype(bf)
    xTa = np.ascontiguousarray(x.T.astype(bf))
    inwT = in_w.T.astype(bf)
    outwT = out_w.T.astype(bf)
    ident = np.eye(128, dtype=np.float32).astype(bf)

    in_maps = []
    for c in range(NC):
        in_maps.append({
            "wc": np.ascontiguousarray(Wd[:, 512 * c : 512 * (c + 1)]),
            "xT": xTa,
            "inwTc": np.ascontiguousarray(inwT[:, 128 * c : 128 * (c + 1)]),
            "inbc": np.ascontiguousarray(in_b[128 * c : 128 * (c + 1), None]),
            "outwTc": np.ascontiguousarray(outwT[:, CLS_M * c : CLS_M * (c + 1)]),
            "outbc": np.ascontiguousarray(out_b[CLS_M * c : CLS_M * (c + 1), None]),
            "ident": ident,
        })
    res = run_bass_kernel_spmd(nc, in_maps, list(range(NC)))
    outT = np.concatenate([res.results[c]["out_t"] for c in range(NC)], axis=0)
    return np.ascontiguousarray(outT.T)


# revision 9
# speedup vs baseline: 1.1519x; 1.1519x over previous
"""Trainium2 Bass kernel for the BasicRNN problem.

Math: the reference's 9 block matmuls per step collapse to
    state_{t+1} = relu(state_t @ W + gate_t * [E, 0, 0]),  state [256, 4096]
with E = x @ in_w.T + in_b, gates at t % 5 == 0, output = O_T @ out_w.T + out_b
where O is the last 1024 state columns.

Device strategy (8 cores, tensor-parallel over W columns):
- Keep the state transposed on-chip: sT = state.T [4096, 256], stored as 32
  row-chunks of 128. W stays stationary on the PE; each step streams sT
  through it: next_sT[rows r] = W[:, r].T @ sT.
- The 4096 state rows are block-permuted so core c owns device blocks
  4c..4c+3 = original chunks {S_c, I_2c, I_2c+1, O_c}. Every core then owns
  one S chunk (balanced E injection) and one O chunk (balanced last step,
  which only needs O). Step 1 contracts only over the 8 S chunks (state_1 is
  [relu(E), 0, 0]).
- Matmul operands are bf16 (fp32 matmul is 4x slower; fp32/f32r weight loads
  are 2x slower). PSUM accumulates fp32. Measured rel err ~7e-3.
- Per step the 4 output blocks are split in two halves; each half is
  AllGathered separately so the collective of one half overlaps the matmuls
  of the other half and of the next step (k-loop runs H0 chunks before H1).
- A tiny throwaway AllGather at program start absorbs the one-time
  communicator-init barrier (~34us) under the weight loads.
- Filler matmuls (identity x E) occupy the PE during collective waits so the
  HAM clock gate stays at 2.4 GHz (idle >3.4us would re-throttle to 1.2).
- State reloads are split across the two HWDGE rings (sync + scalar).
- Final classifier is sharded: core c computes out.T rows [125c, 125c+125)
  with bias; host concatenates and transposes.
"""

import numpy as np

S_DIM, I_DIM, O_DIM = 1024, 2048, 1024
TOTAL = 4096
INPUT_DIM, NUM_CLASSES, BATCH = 2048, 1000, 256
NC = 8
KC = TOTAL // 128          # 32 k-chunks of 128
MPC = KC // NC             # 4 blocks per core
CLS_M = NUM_CLASSES // NC  # 125 classifier rows per core

# device block -> original 128-row chunk of the state vector
BLOCK_PERM = []
for c in range(NC):
    BLOCK_PERM += [c, 8 + 2 * c, 9 + 2 * c, 24 + c]

FILL_WARM = 150   # filler matmuls after the E phase (covers AG0)
FILL_A = 76      # fillers between phase B(s) and phase A(s+1) (covers AG_a)
FILL_B = 24      # fillers between phase A(s) and phase B(s) (covers AG_b)

_cache = {}


def _split_excess_waits(nc, mybir, limit=1, nop_limit=1):
    """This walrus build rejects multiple sync-waits on most instruction
    structs and any wait on Drain. Move excess waits onto preceding
    same-engine nops."""
    counter = [0]

    def make_nop(engine, waits):
        counter[0] += 1
        nop = mybir.InstNoOp(name=f"I-ws{counter[0]}", engine=engine)
        nop.sync_info = mybir.SyncInfo(on_wait=list(waits), on_update=[])
        return nop

    for fn in nc.m.functions:
        for bb in fn.blocks:
            out = []
            changed = False
            for inst in bb.instructions:
                si = getattr(inst, "sync_info", None)
                waits = list(si.on_wait) if si is not None and si.on_wait else []
                lim = 0 if isinstance(inst, mybir.InstDrain) else limit
                if len(waits) > lim:
                    keep = waits[-lim:] if lim else []
                    excess = waits[: len(waits) - lim]
                    for g in range(0, len(excess), nop_limit):
                        nop = make_nop(inst.engine, excess[g : g + nop_limit])
                        nc.register_instruction(nop, overwrite=True)
                        out.append(nop)
                    si.on_wait = keep
                    changed = True
                out.append(inst)
            if changed:
                bb.instructions = out


def _build(T):
    import concourse.bass as bass
    import concourse.tile as tile
    from concourse import mybir

    f32 = mybir.dt.float32
    bf16 = mybir.dt.bfloat16
    B = BATCH

    nc = bass.Bass()
    wc = nc.dram_tensor("wc", [TOTAL, 128 * MPC], bf16, kind="ExternalInput")
    xT = nc.dram_tensor("xT", [INPUT_DIM, B], bf16, kind="ExternalInput")
    inwTc = nc.dram_tensor("inwTc", [INPUT_DIM, 128], bf16, kind="ExternalInput")
    inbc = nc.dram_tensor("inbc", [128, 1], f32, kind="ExternalInput")
    outwTc = nc.dram_tensor("outwTc", [O_DIM, CLS_M], bf16, kind="ExternalInput")
    outbc = nc.dram_tensor("outbc", [CLS_M, 1], f32, kind="ExternalInput")
    ident = nc.dram_tensor("ident", [128, 128], bf16, kind="ExternalInput")
    out_t = nc.dram_tensor("out_t", [CLS_M, B], f32, kind="ExternalOutput")

    S_BLOCKS = [4 * c for c in range(NC)]  # device blocks holding S chunks
    RG = [list(range(NC))]
    ROW_A = {k: 128 * (k // 4) for k in range(KC) if k % 4 == 0}
    ROW_B = {k: 384 * (k // 4) + 128 * (k % 4 - 1) for k in range(KC) if k % 4 >= 1}

    def ag(ins_ap, out_ap):
        nc.gpsimd.collective_compute(
            "AllGather", mybir.AluOpType.bypass,
            replica_groups=RG, ins=[ins_ap], outs=[out_ap],
        )

    with tile.TileContext(nc) as tc:
        with (
            tc.tile_pool(name="wp", bufs=1) as wp,
            tc.tile_pool(name="pers", bufs=1) as pers,
            tc.tile_pool(name="state", bufs=2) as stp,
            tc.tile_pool(name="res", bufs=2) as resp,
            tc.tile_pool(name="psum", bufs=2, space="PSUM") as psp,
            tc.tile_pool(name="psum1", bufs=1, space="PSUM") as psp1,
            tc.tile_pool(name="dram", bufs=2, space="DRAM") as dram,
        ):
            # --- static weights/consts into SBUF (packed single DMAs) ---
            wt = wp.tile([128, KC * 512], bf16, name="wt", tag="wt")
            nc.sync.dma_start(
                wt[:].rearrange("p (k m) -> p k m", m=512),
                wc[:].rearrange("(k p) m -> p k m", p=128),
            )

            def wslice(k, m):  # lhsT tile [128, 128] for (k-chunk, m-block)
                return wt[:, k * 512 + 128 * m : k * 512 + 128 * (m + 1)]

            xt = pers.tile([128, (INPUT_DIM // 128) * B], bf16, name="xt", tag="xt")
            nc.scalar.dma_start(
                xt[:].rearrange("p (k b) -> p k b", b=B),
                xT[:].rearrange("(k p) b -> p k b", p=128),
            )
            iwt = pers.tile([128, INPUT_DIM], bf16, name="iwt", tag="iwt")
            nc.scalar.dma_start(
                iwt[:].rearrange("p (k m) -> p k m", m=128),
                inwTc[:].rearrange("(k p) m -> p k m", p=128),
            )
            id_t = pers.tile([128, 128], bf16, name="ident", tag="ident")
            nc.scalar.dma_start(id_t[:], ident[:])
            inb_t = pers.tile([128, 1], f32, name="inb", tag="inb")
            nc.scalar.dma_start(inb_t[:], inbc[:])
            outb_t = pers.tile([CLS_M, 1], f32, name="outb", tag="outb")
            nc.scalar.dma_start(outb_t[:], outbc[:])
            owt = pers.tile([128, (O_DIM // 128) * CLS_M], bf16, name="owt", tag="owt")
            nc.scalar.dma_start(
                owt[:].rearrange("p (k m) -> p k m", m=CLS_M),
                outwTc[:].rearrange("(k p) m -> p k m", p=128),
            )

            # --- E phase: einj = (in_w @ x.T)[own chunk] + in_b  [128, B] ---
            ps_e = psp.tile([128, B], f32, name="ps0", tag="ps0")
            for k in range(INPUT_DIM // 128):
                nc.tensor.matmul(
                    ps_e[:],
                    iwt[:, 128 * k : 128 * (k + 1)],
                    xt[:, B * k : B * (k + 1)],
                    start=(k == 0), stop=(k == INPUT_DIM // 128 - 1),
                )
            einj = pers.tile([128, B], bf16, name="einj", tag="einj")
            nc.scalar.activation(
                einj[:], ps_e[:], mybir.ActivationFunctionType.Identity, bias=inb_t[:]
            )
            st1_own = resp.tile([128, B], bf16, name="res0", tag="res0")
            nc.scalar.activation(
                st1_own[:], ps_e[:], mybir.ActivationFunctionType.Relu, bias=inb_t[:]
            )
            zero_t = pers.tile([128, B], bf16, name="zero", tag="zero")
            nc.vector.tensor_scalar_mul(zero_t[:], st1_own[:], 0.0)

            ps_d = psp1.tile([128, B], f32, name="psd", tag="psd")

            def fill(n, rhs=None):  # keep PE busy/warm through a collective wait
                # rhs pins the fillers to the producing step: without a
                # data dep the scheduler drains every filler into the first
                # simulated bubble (its collective cost model is ~3x real).
                r = einj if rhs is None else rhs
                for _ in range(n):
                    nc.tensor.matmul(ps_d[:], id_t[:], r[:, 0:B], start=True, stop=True)

            # t0: sT_1 = [relu(E), 0, 0]; step 1 reads only S blocks (all in
            # the A half), so a single half-gather suffices.
            ag_a = dram.tile([128, B], bf16, name="aga", tag="aga")
            nc.sync.dma_start(ag_a[0:128, :], st1_own[:])
            agout_a = dram.tile(
                [1024, B], bf16, name="agouta", tag="agouta", addr_space="Shared"
            )
            ag(ag_a.opt(), agout_a.opt())
            agout_b = None
            fill(FILL_WARM)

            # --- RNN steps s = 1..T-1: sT_{s+1} = relu(W.T @ sT_s [+ inj]) ---
            last = T - 1
            res = {}
            for s in range(1, T):
                active = S_BLOCKS if s == 1 else list(range(KC))
                m_list = [MPC - 1] if s == last else list(range(MPC))
                act_a = [k for k in active if k % 4 == 0]
                act_b = [k for k in active if k % 4 >= 1]
                inject = s % 5 == 0

                st = {}
                for j, k in enumerate(act_a):
                    st[k] = stp.tile([128, B], bf16, name=f"st{k}", tag=f"st{k}")
                    eng = nc.sync if j % 2 == 0 else nc.scalar
                    eng.dma_start(st[k][:], agout_a[ROW_A[k] : ROW_A[k] + 128, :])
                for j, k in enumerate(act_b):
                    st[k] = stp.tile([128, B], bf16, name=f"st{k}", tag=f"st{k}")
                    eng = nc.sync if j % 2 == 0 else nc.scalar
                    eng.dma_start(st[k][:], agout_b[ROW_B[k] : ROW_B[k] + 128, :])

                # phase A: H0 contraction for every output block
                ps = {}
                for m in m_list:
                    pool = psp1 if m == MPC - 1 else psp
                    ps[m] = pool.tile([128, B], f32, name=f"ps{m}", tag=f"ps{m}")
                    for i, k in enumerate(act_a):
                        nc.tensor.matmul(
                            ps[m][:],
                            wslice(k, m),
                            st[k][:],
                            start=(i == 0),
                            stop=(not act_b)
                            and i == len(act_a) - 1
                            and not (inject and m == 0),
                        )
                if act_b:
                    fill(FILL_B, rhs=st[act_a[-1]])
                # phase B: H1 contraction, relu, shard out, half-gathers
                res = {}
                for m in m_list:
                    for i, k in enumerate(act_b):
                        nc.tensor.matmul(
                            ps[m][:],
                            wslice(k, m),
                            st[k][:],
                            start=False,
                            stop=i == len(act_b) - 1 and not (inject and m == 0),
                        )
                    if inject and m == 0:
                        nc.tensor.matmul(
                            ps[m][:], id_t[:], einj[:], start=False, stop=True
                        )
                    res[m] = resp.tile([128, B], bf16, name=f"res{m}", tag=f"res{m}")
                    nc.vector.tensor_relu(res[m][:], ps[m][:])
                    if s == last:
                        continue
                    if m == 0:
                        new_a = dram.tile([128, B], bf16, name="aga", tag="aga")
                        nc.sync.dma_start(new_a[0:128, :], res[0][:])
                        agout_a = dram.tile(
                            [1024, B], bf16, name="agouta", tag="agouta",
                            addr_space="Shared",
                        )
                        ag(new_a.opt(), agout_a.opt())
                    if m == 3:
                        new_b = dram.tile([384, B], bf16, name="agb", tag="agb")
                        nc.sync.dma_start(new_b[0:128, :], res[1][:])
                        nc.sync.dma_start(new_b[128:256, :], res[2][:])
                        nc.sync.dma_start(new_b[256:384, :], res[3][:])
                        agout_b = dram.tile(
                            [3072, B], bf16, name="agoutb", tag="agoutb",
                            addr_space="Shared",
                        )
                        ag(new_b.opt(), agout_b.opt())
                if s != last:
                    fill(FILL_A, rhs=res[MPC - 1])

            # --- gather O chunks, classifier slice, bias, out ---
            ago_in = dram.tile([128, B], bf16, name="agoin", tag="agoin")
            nc.sync.dma_start(ago_in[:], res[MPC - 1][:])
            ago_out = dram.tile(
                [O_DIM, B], bf16, name="agoout", tag="agoout", addr_space="Shared"
            )
            ag(ago_in.opt(), ago_out.opt())
            fill(FILL_A, rhs=res[MPC - 1])
            ot = {}
            for k in range(O_DIM // 128):
                ot[k] = stp.tile([128, B], bf16, name=f"ot{k}", tag=f"st{k}")
                eng = nc.sync if k % 2 == 0 else nc.scalar
                eng.dma_start(ot[k][:], ago_out[128 * k : 128 * (k + 1), :])
            ps_c = psp.tile([CLS_M, B], f32, name="ps1", tag="ps1")
            for k in range(O_DIM // 128):
                nc.tensor.matmul(
                    ps_c[:], owt[:, CLS_M * k : CLS_M * (k + 1)], ot[k][:],
                    start=(k == 0), stop=(k == O_DIM // 128 - 1),
                )
            out_sb = pers.tile([CLS_M, B], f32, name="outsb", tag="outsb")
            nc.scalar.activation(
                out_sb[:], ps_c[:], mybir.ActivationFunctionType.Identity,
                bias=outb_t[:],
            )
            nc.sync.dma_start(out_t[:], out_sb[:])

    _split_excess_waits(nc, mybir)
    return nc


def kernel(x, W, in_w, in_b, out_w, out_b, time_steps):
    T = int(time_steps)
    x = np.ascontiguousarray(x, dtype=np.float32)
    W = np.ascontiguousarray(W, dtype=np.float32)
    in_w = np.ascontiguousarray(in_w, dtype=np.float32)
    in_b = np.ascontiguousarray(in_b, dtype=np.float32)
    out_w = np.ascontiguousarray(out_w, dtype=np.float32)
    out_b = np.ascontiguousarray(out_b, dtype=np.float32)

    if T < 2:
        # T=0: O stays 0; T=1: state_1 = [relu(E),0,0], O still 0.
        return np.broadcast_to(out_b, (BATCH, NUM_CLASSES)).astype(np.float32).copy()

    import ml_dtypes
    from concourse.bass_utils import run_bass_kernel_spmd

    if T not in _cache:
        _cache[T] = _build(T)
    nc = _cache[T]

    bf = ml_dtypes.bfloat16
    # block-permute W rows and columns to the device layout
    Wd = W.reshape(KC, 128, TOTAL)[BLOCK_PERM].reshape(TOTAL, TOTAL)
    Wd = Wd.reshape(TOTAL, KC, 128)[:, BLOCK_PERM].reshape(TOTAL, TOTAL)
    Wd = Wd.astype(bf)
    xTa = np.ascontiguousarray(x.T.astype(bf))
    inwT = in_w.T.astype(bf)
    outwT = out_w.T.astype(bf)
    ident = np.eye(128, dtype=np.float32).astype(bf)

    in_maps = []
    for c in range(NC):
        in_maps.append({
            "wc": np.ascontiguousarray(Wd[:, 512 * c : 512 * (c + 1)]),
            "xT": xTa,
            "inwTc": np.ascontiguousarray(inwT[:, 128 * c : 128 * (c + 1)]),
            "inbc": np.ascontiguousarray(in_b[128 * c : 128 * (c + 1), None]),
            "outwTc": np.ascontiguousarray(outwT[:, CLS_M * c : CLS_M * (c + 1)]),
            "outbc": np.ascontiguousarray(out_b[CLS_M * c : CLS_M * (c + 1), None]),
            "ident": ident,
        })
    res = run_bass_kernel_spmd(nc, in_maps, list(range(NC)))
    outT = np.concatenate([res.results[c]["out_t"] for c in range(NC)], axis=0)
    return np.ascontiguousarray(outT.T)
